# revision 17
# baseline (speedup 1.0000x reference)
"""Trainium2 Bass kernel for nn_ClosedFlyLoop (8 NeuronCores, W-sharded).

Strategy (v4)
-------------
- W (AP) axis sharded across 8 cores; halos handled on the host during
  sharding, so all device work is local.
- m_0 (global mean of |m|) is computed on the HOST and shipped as two
  per-partition scalars (sU=-0.5/m_0, sH=0.25/m_0): removes the AllReduce
  (15+us fixed cost) and its serialization.
- All small constants (Toeplitz pieces + identity matrices) are packed into
  ONE DMA; input planes are loaded m/v-first so both the m-chain and the
  first convolutions start ~2us in.
- 14 Gaussian-derivative convolutions run on TensorE as banded-Toeplitz
  matmuls into full-plane [128,2048] PSUM tiles.
- The advection sign is folded into downstream subtracts / negated-identity
  matmuls, so v0/v1 are used unnegated (two fewer ops, one less plane).
- Elementwise 2x2 algebra is spread across engines by measured cost:
    * DVE: bf16 tensor_tensor at 2 elem/cycle (1127ns/plane-op).
    * Pool (gpsimd): ~11 independent tensor_tensor ops (4158ns each) chosen
      OFF the critical tail (side products + per-channel advection t2).
    * ACT: unary-affine ops + PSUM->SBUF copies of conv outputs.
    * PE: composes o00/o11 as sums of product planes via identity-matmul
      PSUM accumulation (214ns per [128,512] slice) after the convs release
      PSUM; ACT copies the sums back as bf16 for the output DMA.

Layout: each [1024, 256] per-core field is a "plane" [128, 2048] with
plane[p, a*256 + w] = field[a*128 + p, w]  (a = h-chunk index).
"""

import numpy as np
import ml_dtypes
from contextlib import ExitStack

import concourse.bass as bass
import concourse.bacc as bacc
import concourse.tile as tile
from concourse import mybir
from concourse.bass_utils import run_bass_kernel_spmd

F32 = mybir.dt.float32
BF16 = mybir.dt.bfloat16
AF = mybir.ActivationFunctionType
OP = mybir.AluOpType

H, W = 1024, 2048
NCORES = 8
WS = W // NCORES          # 256 columns per core
NCH = 8                   # h-chunks per core
P = 128
FD = NCH * WS             # 2048 plane free dim
R = 20                    # halo / pad radius
SIGMA = 5
D_AP = 2.27
D_DV = 2.27

_BF = ml_dtypes.bfloat16

# const pack layout (columns of the single [128, 1408] bf16 const tensor)
_C_TY = 0          # 3 x 128 cols  (tyA tyB tyC)
_C_TX = 384        # 2 x 256 cols  (txA txB)
_C_TXC = 896       # 256 cols      (txC padded to 128 partitions)
_C_ID = 1152       # 128 cols      identity
_C_NID = 1280      # 128 cols      negated identity
_C_COLS = 1408


def _gauss_d1_kernel():
    x = np.arange(-R, R + 1).astype(np.float64)
    phi = np.exp(-0.5 / SIGMA ** 2 * x ** 2)
    phi = phi / phi.sum()
    return ((-x / SIGMA ** 2) * phi).astype(np.float32)


def _build_consts():
    K = _gauss_d1_kernel().astype(np.float64)
    Ky = K / D_DV
    Kx = K / D_AP
    k = np.arange(128)[:, None]
    m = np.arange(128)[None, :]
    x = np.arange(WS)[None, :]

    def band(tmat, idx, taps):
        ok = (idx >= 0) & (idx <= 40)
        tmat[ok] = taps[idx[ok]]
        return tmat

    tyA = band(np.zeros((128, 128)), k - m - 108, Ky)
    tyB = band(np.zeros((128, 128)), k - m + 20, Ky)
    tyC = band(np.zeros((128, 128)), k - m + 148, Ky)
    txA = band(np.zeros((128, WS)), k - x, Kx)
    txB = band(np.zeros((128, WS)), 128 + k - x, Kx)
    txC = band(np.zeros((128, WS)), 256 + np.arange(128)[:, None] - x, Kx)
    ident = np.eye(128)
    pack = np.concatenate(
        [tyA, tyB, tyC, txA, txB, txC, ident, -ident], axis=1)
    assert pack.shape == (128, _C_COLS)
    return pack.astype(_BF)


def _to_plane(arr):
    """[..., H, WS] -> [..., 128, FD]"""
    sh = arr.shape[:-2]
    return (arr.reshape(*sh, NCH, P, WS).swapaxes(-3, -2)
            .reshape(*sh, P, FD))


def _from_plane(pl):
    sh = pl.shape[:-2]
    return (pl.reshape(*sh, P, NCH, WS).swapaxes(-3, -2)
            .reshape(*sh, H, WS))


# ---------------------------------------------------------------------------
# device kernel builder
# ---------------------------------------------------------------------------

def _build(nc, cad, myo):
    yn_h = nc.dram_tensor("yn", [8, P, FD], BF16, kind="ExternalInput")
    yt_h = nc.dram_tensor("yt", [7, WS + 2 * R, H], BF16, kind="ExternalInput")
    ct_h = nc.dram_tensor("ct", [P, _C_COLS], BF16, kind="ExternalInput")
    sc_h = nc.dram_tensor("sc", [P, 2], F32, kind="ExternalInput")
    out_h = nc.dram_tensor("out", [5, P, FD], BF16, kind="ExternalOutput")

    with tile.TileContext(nc, num_cores=NCORES) as tc:
        with ExitStack() as ctx:
            _body(ctx, tc, yn_h, yt_h, ct_h, sc_h, out_h, cad, myo)
    return nc


def _body(ctx, tc, yn_h, yt_h, ct_h, sc_h, out_h, cad, myo):
    nc = tc.nc
    cad0, cad1, cad2 = cad
    myo0, myo1, myo2, myo3, myo4 = myo

    pln = ctx.enter_context(tc.tile_pool(name="pln", bufs=1))
    lng = ctx.enter_context(tc.tile_pool(name="lng", bufs=1))
    ytp = ctx.enter_context(tc.tile_pool(name="ytp", bufs=1))
    tmpp = ctx.enter_context(tc.tile_pool(name="tmpp", bufs=4))
    outp = ctx.enter_context(tc.tile_pool(name="outp", bufs=1))
    ps = ctx.enter_context(tc.tile_pool(name="ps", bufs=1, space="PSUM"))
    tiny = ctx.enter_context(tc.tile_pool(name="tiny", bufs=1))

    def plane(name, pool=None, dt=BF16, tag=None, bufs=None):
        pool = pool if pool is not None else tmpp
        tag = tag if tag is not None else ("tmp" if pool is tmpp else name)
        return pool.tile([P, FD], dt, tag=tag, name=name, bufs=bufs)

    def longp(name, dt=BF16):
        return plane(name, lng, dt)

    TS = nc.vector.tensor_scalar
    ACT = nc.scalar.activation

    def vadd(out, a, b):
        nc.vector.tensor_add(out, a, b)

    def vsub(out, a, b):
        nc.vector.tensor_sub(out, a, b)

    def vmul(out, a, b):
        nc.vector.tensor_mul(out, a, b)

    def pmul(out, a, b):
        nc.gpsimd.tensor_mul(out, a, b)

    def psub(out, a, b):
        nc.gpsimd.tensor_sub(out, a, b)

    # ---------------- loads (order matters: SP queue is in-order) ----------
    ct = pln.tile([P, _C_COLS], BF16, tag="ct", name="ct")
    nc.sync.dma_start(out=ct, in_=ct_h[:])
    ty = [ct[:, _C_TY + 128 * j:_C_TY + 128 * (j + 1)] for j in range(3)]
    tx = [ct[:, _C_TX:_C_TX + WS], ct[:, _C_TX + WS:_C_TX + 2 * WS],
          ct[0:40, _C_TXC:_C_TXC + WS]]
    ident = ct[:, _C_ID:_C_ID + 128]
    nident = ct[:, _C_NID:_C_NID + 128]

    yn = [None] * 8

    def load(c, pool, tag=None):
        t = plane(f"yn{c}", pool, tag=tag, bufs=2 if tag else None)
        nc.sync.dma_start(out=t, in_=yn_h[c])
        yn[c] = t

    load(1, pln)            # m01 (squares first)
    load(2, pln)            # m10
    load(0, pln)            # m00
    load(3, pln)            # m11
    load(5, tmpp, "vv")     # v0  (conv 5)
    load(6, tmpp, "vv")     # v1

    # ---------------- conv machinery ----------------
    def conv_channel(c, on_Y, on_X):
        ytt = [ytp.tile([128, H], BF16, tag="ytk0", name=f"ytk0_{c}"),
               ytp.tile([128, H], BF16, tag="ytk1", name=f"ytk1_{c}"),
               ytp.tile([40, H], BF16, tag="ytk2", name=f"ytk2_{c}")]
        nc.sync.dma_start(out=ytt[0], in_=yt_h[c, 0:128])
        nc.sync.dma_start(out=ytt[1], in_=yt_h[c, 128:256])
        nc.sync.dma_start(out=ytt[2], in_=yt_h[c, 256:296])
        psY = ps.tile([P, FD], F32, tag="psY", name=f"psY{c}")
        psX = ps.tile([P, FD], F32, tag="psX", name=f"psX{c}")
        for i in range(NCH):
            osl = slice(i * WS, (i + 1) * WS)
            for kp in range(3):
                src = (i - 1 + kp) % NCH
                nc.tensor.matmul(
                    psY[:, osl], lhsT=ty[kp],
                    rhs=yn[c][:, src * WS:(src + 1) * WS],
                    start=(kp == 0), stop=(kp == 2))
        on_Y(psY)
        for i in range(NCH):
            osl = slice(i * WS, (i + 1) * WS)
            hsl = slice(i * P, (i + 1) * P)
            for kp in range(3):
                nc.tensor.matmul(
                    psX[:, osl], lhsT=ytt[kp][:, hsl], rhs=tx[kp],
                    start=(kp == 0), stop=(kp == 2))
        on_X(psX)

    def copy_to(dst):
        def f(psrc):
            nc.scalar.copy(dst, psrc)
        return f

    # ---------------- early DVE (dep: m planes) ----------------------------
    m01, m10, m00, m11 = yn[1], yn[2], yn[0], yn[3]
    S = longp("S")
    nc.gpsimd.tensor_add(S, m01, m10)    # Pool
    D = longp("D")
    vsub(D, m00, m11)
    T = longp("T")
    nc.gpsimd.tensor_add(T, m00, m11)    # Pool

    # ---------------- early ACT (squares feed the G2 critical path) --------
    m01sq = plane("m01sq")
    ACT(m01sq, m01, AF.Square)
    m10sq = plane("m10sq")
    ACT(m10sq, m10, AF.Square)
    hD2 = plane("hD2")
    ACT(hD2, D, AF.Square, scale=float(np.sqrt(0.5)))

    s_a = plane("s_a")
    vadd(s_a, m01sq, hD2)
    G2 = plane("G2")
    vadd(G2, s_a, m10sq)

    sc_t = tiny.tile([P, 2], F32, tag="sc", name="sc")
    nc.sync.dma_start(out=sc_t, in_=sc_h[:])
    sU_vec = sc_t[:, 0:1]
    sH_vec = sc_t[:, 1:2]

    g = plane("g", tag="g", bufs=1, dt=F32)
    ACT(g, G2, AF.Sqrt)
    rsG = plane("rsG", tag="rsG", bufs=1, dt=F32)
    nc.vector.reciprocal_approx_fast(out=rsG, in_=g)
    # ---------------- conv 5 (v0) ------------------------------------------
    E00 = longp("E00")
    Xv0 = plane("Xv0")
    conv_channel(5, copy_to(E00), copy_to(Xv0))

    # remaining input planes (issued on SP after conv5's yt loads)
    load(4, pln)    # c field
    load(7, pln)    # gamma (pre-scaled by cad2 on host)

    # ---------------- conv 6 (v1) ------------------------------------------
    Yv1 = plane("Yv1")
    E11 = longp("E11")
    conv_channel(6, copy_to(Yv1), copy_to(E11))

    u = plane("u", tag="u", bufs=1)
    ACT(u, g, AF.Copy, bias=1.0, scale=sU_vec)
    cfld = yn[4]
    cc = longp("cc")
    TS(cc, cfld, -myo2, myo1, OP.mult, OP.add)

    # ---------------- velocity combos --------------------------------------
    W2 = plane("W2")
    vsub(W2, Yv1, Xv0)
    A2p = longp("A2p")
    vadd(A2p, Xv0, Yv1)
    trE = longp("trE")
    vadd(trE, E00, E11)
    Bm = plane("Bm")
    vsub(Bm, E00, E11)
    hW2 = plane("hW2")
    ACT(hW2, W2, AF.Copy, scale=0.5)
    mt = longp("mt")
    ACT(mt, T, AF.Copy, scale=float(myo3))

    Q2 = longp("Q2")
    vmul(Q2, hW2, D)
    So = longp("So")
    pmul(So, hW2, S)                      # Pool
    p1 = plane("p1")
    vmul(p1, D, Bm)
    p2 = plane("p2")
    vmul(p2, A2p, S)
    devE2 = plane("devE2")
    vadd(devE2, p1, p2)
    habs = plane("habs")
    ACT(habs, devE2, AF.Abs, scale=sH_vec)
    h = plane("h")
    vmul(h, habs, rsG)
    uc = longp("uc")
    vmul(uc, u, cc)
    hc = longp("hc")
    vmul(hc, h, cc)
    trEb = plane("trEb")
    TS(trEb, trE, cad1, cad0, OP.mult, OP.subtract)

    # ---------------- c-channel conv + cdot --------------------------------
    # advection terms are computed UNNEGATED (t3 = v0*sY + v1*sX); the sign
    # is folded into the consuming subtract / negated-identity compose.
    sY4 = plane("sY4", tag="scpy", bufs=3)
    sX4 = plane("sX4", tag="scpy", bufs=3)
    conv_channel(4, copy_to(sY4), copy_to(sX4))
    t1_4 = plane("t1_4", tag="advt", bufs=3)
    vmul(t1_4, yn[5], sY4)
    t2_4 = plane("t2_4", tag="advt", bufs=3)
    pmul(t2_4, yn[6], sX4)                # Pool
    adv4 = plane("adv4", tag="t3", bufs=2)
    vadd(adv4, t1_4, t2_4)
    w1c = plane("w1c")
    vmul(w1c, cfld, trEb)
    sc1 = plane("sc1")
    vsub(sc1, w1c, adv4)
    cdot = outp.tile([P, FD], BF16, tag="out", name="cdot", bufs=2)
    vadd(cdot, sc1, yn[7])

    # ---------------- phase-B scalars --------------------------------------
    u2 = longp("u2")
    TS(u2, uc, 2.0, None, OP.mult)
    huc = plane("huc", tag="huc", bufs=1)
    TS(huc, uc, 0.5, None, OP.mult)
    trEpc = plane("trEpc")
    pmul(trEpc, uc, trE)                  # Pool
    rq = longp("rq")
    hcD = plane("hcD")
    pmul(hcD, hc, D)                      # Pool
    q = longp("q")
    TS(q, T, myo4, myo0, OP.mult, OP.subtract)
    vadd(rq, trEpc, q)
    qp = plane("qp")
    vadd(qp, hcD, q)
    qm = plane("qm")
    vsub(qm, q, hcD)
    a00 = plane("a00")
    pmul(a00, u2, E00)                    # Pool
    r00 = longp("r00")
    vadd(r00, a00, qp)
    a11 = plane("a11")
    pmul(a11, u2, E11)                    # Pool
    r11 = longp("r11")
    vadd(r11, a11, qm)
    ucA = plane("ucA")
    vmul(ucA, huc, A2p)
    hm01 = plane("hm01")
    vmul(hm01, hc, m01)
    Epc01 = longp("Epc01")
    vadd(Epc01, ucA, hm01)
    hm10 = plane("hm10")
    vmul(hm10, hc, m10)
    Epc10 = longp("Epc10")
    vadd(Epc10, ucA, hm10)

    def adv_copy(c, t3, t2mul):
        sY = plane(f"sY{c}", tag="scpy", bufs=3)
        sX = plane(f"sX{c}", tag="scpy", bufs=3)
        conv_channel(c, copy_to(sY), copy_to(sX))
        t1 = plane(f"t1_{c}", tag="advt", bufs=3)
        vmul(t1, yn[5], sY)
        t2 = plane(f"t2_{c}", tag="advt", bufs=3)
        t2mul(t2, yn[6], sX)
        vadd(t3, t1, t2)

    # ---------------- m-channel 1 -> o01 -----------------------------------
    t3_1 = plane("t3_1", tag="t3", bufs=2)
    adv_copy(1, t3_1, pmul)
    w1 = plane("w1")
    vmul(w1, Epc01, T)
    mq01 = plane("mq01")
    pmul(mq01, m01, rq)                   # Pool
    preA1 = plane("preA1")
    vsub(preA1, Q2, t3_1)
    s01 = plane("s01")
    vadd(s01, w1, mq01)
    o01 = outp.tile([P, FD], BF16, tag="out", name="o01", bufs=2)
    vadd(o01, s01, preA1)

    # ---------------- m-channel 2 -> o10 -----------------------------------
    t3_2 = plane("t3_2", tag="t3", bufs=2)
    adv_copy(2, t3_2, pmul)
    w3 = plane("w3")
    pmul(w3, Epc10, T)                    # Pool
    mq10 = plane("mq10")
    pmul(mq10, m10, rq)                   # Pool
    preA2 = plane("preA2")
    vsub(preA2, Q2, t3_2)
    s10 = plane("s10")
    vadd(s10, w3, mq10)
    o10 = outp.tile([P, FD], BF16, tag="out", name="o10", bufs=2)
    vadd(o10, s10, preA2)

    # ---------------- m-channel 0 -> o00 -----------------------------------
    t3_0 = plane("t3_0", tag="t3", bufs=2)
    adv_copy(0, t3_0, vmul)
    x1 = plane("x1")
    vmul(x1, m01, Epc10)
    x2 = plane("x2")
    vmul(x2, m10, Epc01)
    X = plane("X", tag="X", bufs=1)
    vadd(X, x1, x2)
    mm00 = plane("mm00")
    vmul(mm00, m00, r00)

    # ---------------- m-channel 3 -> o11 -----------------------------------
    t3_3 = plane("t3_3", tag="t3", bufs=2)
    adv_copy(3, t3_3, vmul)
    mm11 = plane("mm11")
    vmul(mm11, m11, r11)

    # o00 = mm00 + X + (mt - t3_0 - So);  o11 = mm11 + X + (So - t3_3)
    e1 = plane("e1")
    vsub(e1, mt, t3_0)
    e2 = plane("e2")
    vsub(e2, e1, So)
    t00 = plane("t00")
    vadd(t00, mm00, X)
    o00 = outp.tile([P, FD], BF16, tag="out", name="o00", bufs=2)
    vadd(o00, t00, e2)
    f1 = plane("f1")
    vsub(f1, So, t3_3)
    t11 = plane("t11")
    vadd(t11, mm11, X)
    o11 = outp.tile([P, FD], BF16, tag="out", name="o11", bufs=2)
    vadd(o11, t11, f1)

    # output DMAs last on the SP queue: all input issues precede them
    nc.sync.dma_start(out=out_h[4], in_=cdot)
    nc.sync.dma_start(out=out_h[1], in_=o01)
    nc.sync.dma_start(out=out_h[2], in_=o10)
    nc.sync.dma_start(out=out_h[0], in_=o00)
    nc.sync.dma_start(out=out_h[3], in_=o11)


# ---------------------------------------------------------------------------
# host entry point
# ---------------------------------------------------------------------------

_CACHE = {}


def _get_nc(cad, myo):
    key = (tuple(np.asarray(cad, np.float64).tolist()),
           tuple(np.asarray(myo, np.float64).tolist()))
    if key not in _CACHE:
        nc = bacc.Bacc("TRN2", target_bir_lowering=False, debug=False,
                       num_devices=NCORES)
        _build(nc, *key)
        nc.compile()
        _CACHE[key] = nc
    return _CACHE[key]


def _make_in_maps(y, v, gamma_ds, cad):
    all7 = np.concatenate([y, v], axis=0).astype(np.float32)   # [7, H, W]
    ypad = np.pad(all7, ((0, 0), (0, 0), (R, R)), mode="reflect")
    ct = _build_consts()

    m4 = y[:4].astype(np.float32)
    m_norm = np.sqrt(np.sum(m4.astype(np.float64) ** 2, axis=0))
    m0 = float(np.mean(m_norm))
    sc = np.tile(np.array([[-0.5 / m0, 0.25 / m0]], np.float32), (P, 1))

    gam_s = (cad[2] * gamma_ds).astype(np.float32)
    all8 = np.concatenate([all7, gam_s[None]], axis=0)

    in_maps = []
    for core in range(NCORES):
        w0 = core * WS
        yn = _to_plane(all8[:, :, w0:w0 + WS]).astype(_BF)
        yt = np.ascontiguousarray(
            ypad[:, :, w0:w0 + WS + 2 * R].transpose(0, 2, 1)).astype(_BF)
        in_maps.append({"yn": yn, "yt": yt, "ct": ct, "sc": sc})
    return in_maps


def kernel(y, v, gamma_ds, cad_coefs, myo_coefs):
    y = np.asarray(y, np.float32)
    v = np.asarray(v, np.float32)
    gamma_ds = np.asarray(gamma_ds, np.float32)
    cad = np.maximum(np.asarray(cad_coefs, np.float32), 0)
    myo = np.maximum(np.asarray(myo_coefs, np.float32), 0)

    nc = _get_nc(cad, myo)
    in_maps = _make_in_maps(y, v, gamma_ds, cad)
    res = run_bass_kernel_spmd(nc, in_maps, core_ids=list(range(NCORES)))
    outs = [_from_plane(res.results[c]["out"].astype(np.float32))
            for c in range(NCORES)]
    return np.concatenate(outs, axis=-1)


# revision 18
# speedup vs baseline: 1.0025x; 1.0025x over previous
"""Trainium2 Bass kernel for nn_ClosedFlyLoop (8 NeuronCores, W-sharded).

Strategy (v4)
-------------
- W (AP) axis sharded across 8 cores; halos handled on the host during
  sharding, so all device work is local.
- m_0 (global mean of |m|) is computed on the HOST and shipped as two
  per-partition scalars (sU=-0.5/m_0, sH=0.25/m_0): removes the AllReduce
  (15+us fixed cost) and its serialization.
- All small constants (Toeplitz pieces + identity matrices) are packed into
  ONE DMA; input planes are loaded m/v-first so both the m-chain and the
  first convolutions start ~2us in.
- 14 Gaussian-derivative convolutions run on TensorE as banded-Toeplitz
  matmuls into full-plane [128,2048] PSUM tiles.
- The advection sign is folded into downstream subtracts / negated-identity
  matmuls, so v0/v1 are used unnegated (two fewer ops, one less plane).
- Elementwise 2x2 algebra is spread across engines by measured cost:
    * DVE: bf16 tensor_tensor at 2 elem/cycle (1127ns/plane-op).
    * Pool (gpsimd): ~11 independent tensor_tensor ops (4158ns each) chosen
      OFF the critical tail (side products + per-channel advection t2).
    * ACT: unary-affine ops + PSUM->SBUF copies of conv outputs.
    * PE: composes o00/o11 as sums of product planes via identity-matmul
      PSUM accumulation (214ns per [128,512] slice) after the convs release
      PSUM; ACT copies the sums back as bf16 for the output DMA.

Layout: each [1024, 256] per-core field is a "plane" [128, 2048] with
plane[p, a*256 + w] = field[a*128 + p, w]  (a = h-chunk index).
"""

import numpy as np
import ml_dtypes
from contextlib import ExitStack

import concourse.bass as bass
import concourse.bacc as bacc
import concourse.tile as tile
from concourse import mybir
from concourse.bass_utils import run_bass_kernel_spmd

F32 = mybir.dt.float32
BF16 = mybir.dt.bfloat16
AF = mybir.ActivationFunctionType
OP = mybir.AluOpType

H, W = 1024, 2048
NCORES = 8
WS = W // NCORES          # 256 columns per core
NCH = 8                   # h-chunks per core
P = 128
FD = NCH * WS             # 2048 plane free dim
R = 20                    # halo / pad radius
SIGMA = 5
D_AP = 2.27
D_DV = 2.27

_BF = ml_dtypes.bfloat16

# const pack layout (columns of the single [128, 1408] bf16 const tensor)
_C_TY = 0          # 3 x 128 cols  (tyA tyB tyC)
_C_TX = 384        # 2 x 256 cols  (txA txB)
_C_TXC = 896       # 256 cols      (txC padded to 128 partitions)
_C_ID = 1152       # 128 cols      identity
_C_NID = 1280      # 128 cols      negated identity
_C_COLS = 1408


def _gauss_d1_kernel():
    x = np.arange(-R, R + 1).astype(np.float64)
    phi = np.exp(-0.5 / SIGMA ** 2 * x ** 2)
    phi = phi / phi.sum()
    return ((-x / SIGMA ** 2) * phi).astype(np.float32)


def _build_consts():
    K = _gauss_d1_kernel().astype(np.float64)
    Ky = K / D_DV
    Kx = K / D_AP
    k = np.arange(128)[:, None]
    m = np.arange(128)[None, :]
    x = np.arange(WS)[None, :]

    def band(tmat, idx, taps):
        ok = (idx >= 0) & (idx <= 40)
        tmat[ok] = taps[idx[ok]]
        return tmat

    tyA = band(np.zeros((128, 128)), k - m - 108, Ky)
    tyB = band(np.zeros((128, 128)), k - m + 20, Ky)
    tyC = band(np.zeros((128, 128)), k - m + 148, Ky)
    txA = band(np.zeros((128, WS)), k - x, Kx)
    txB = band(np.zeros((128, WS)), 128 + k - x, Kx)
    txC = band(np.zeros((128, WS)), 256 + np.arange(128)[:, None] - x, Kx)
    ident = np.eye(128)
    pack = np.concatenate(
        [tyA, tyB, tyC, txA, txB, txC, ident, -ident], axis=1)
    assert pack.shape == (128, _C_COLS)
    return pack.astype(_BF)


def _to_plane(arr):
    """[..., H, WS] -> [..., 128, FD]"""
    sh = arr.shape[:-2]
    return (arr.reshape(*sh, NCH, P, WS).swapaxes(-3, -2)
            .reshape(*sh, P, FD))


def _from_plane(pl):
    sh = pl.shape[:-2]
    return (pl.reshape(*sh, P, NCH, WS).swapaxes(-3, -2)
            .reshape(*sh, H, WS))


# ---------------------------------------------------------------------------
# device kernel builder
# ---------------------------------------------------------------------------

def _build(nc, cad, myo):
    yn_h = nc.dram_tensor("yn", [8, P, FD], BF16, kind="ExternalInput")
    yt_h = nc.dram_tensor("yt", [7, WS + 2 * R, H], BF16, kind="ExternalInput")
    ct_h = nc.dram_tensor("ct", [P, _C_COLS], BF16, kind="ExternalInput")
    sc_h = nc.dram_tensor("sc", [P, 2], F32, kind="ExternalInput")
    out_h = nc.dram_tensor("out", [5, P, FD], BF16, kind="ExternalOutput")

    with tile.TileContext(nc, num_cores=NCORES) as tc:
        with ExitStack() as ctx:
            _body(ctx, tc, yn_h, yt_h, ct_h, sc_h, out_h, cad, myo)
    return nc


def _body(ctx, tc, yn_h, yt_h, ct_h, sc_h, out_h, cad, myo):
    nc = tc.nc
    cad0, cad1, cad2 = cad
    myo0, myo1, myo2, myo3, myo4 = myo

    pln = ctx.enter_context(tc.tile_pool(name="pln", bufs=1))
    lng = ctx.enter_context(tc.tile_pool(name="lng", bufs=1))
    ytp = ctx.enter_context(tc.tile_pool(name="ytp", bufs=1))
    tmpp = ctx.enter_context(tc.tile_pool(name="tmpp", bufs=4))
    outp = ctx.enter_context(tc.tile_pool(name="outp", bufs=1))
    ps = ctx.enter_context(tc.tile_pool(name="ps", bufs=1, space="PSUM"))
    tiny = ctx.enter_context(tc.tile_pool(name="tiny", bufs=1))

    def plane(name, pool=None, dt=BF16, tag=None, bufs=None):
        pool = pool if pool is not None else tmpp
        tag = tag if tag is not None else ("tmp" if pool is tmpp else name)
        return pool.tile([P, FD], dt, tag=tag, name=name, bufs=bufs)

    def longp(name, dt=BF16):
        return plane(name, lng, dt)

    TS = nc.vector.tensor_scalar
    ACT = nc.scalar.activation

    def vadd(out, a, b):
        nc.vector.tensor_add(out, a, b)

    def vsub(out, a, b):
        nc.vector.tensor_sub(out, a, b)

    def vmul(out, a, b):
        nc.vector.tensor_mul(out, a, b)

    def pmul(out, a, b):
        nc.gpsimd.tensor_mul(out, a, b)

    def psub(out, a, b):
        nc.gpsimd.tensor_sub(out, a, b)

    # ---------------- loads (order matters: SP queue is in-order) ----------
    ct = pln.tile([P, _C_COLS], BF16, tag="ct", name="ct")
    nc.sync.dma_start(out=ct, in_=ct_h[:])
    ty = [ct[:, _C_TY + 128 * j:_C_TY + 128 * (j + 1)] for j in range(3)]
    tx = [ct[:, _C_TX:_C_TX + WS], ct[:, _C_TX + WS:_C_TX + 2 * WS],
          ct[0:40, _C_TXC:_C_TXC + WS]]
    ident = ct[:, _C_ID:_C_ID + 128]
    nident = ct[:, _C_NID:_C_NID + 128]

    yn = [None] * 8

    def load(c, pool, tag=None):
        t = plane(f"yn{c}", pool, tag=tag, bufs=2 if tag else None)
        nc.sync.dma_start(out=t, in_=yn_h[c])
        yn[c] = t

    load(1, pln)            # m01 (squares first)
    load(2, pln)            # m10
    load(0, pln)            # m00
    load(3, pln)            # m11
    load(5, tmpp, "vv")     # v0  (conv 5)
    load(6, tmpp, "vv")     # v1

    # ---------------- conv machinery ----------------
    def conv_channel(c, on_Y, on_X):
        ytt = [ytp.tile([128, H], BF16, tag="ytk0", name=f"ytk0_{c}"),
               ytp.tile([128, H], BF16, tag="ytk1", name=f"ytk1_{c}"),
               ytp.tile([40, H], BF16, tag="ytk2", name=f"ytk2_{c}")]
        nc.sync.dma_start(out=ytt[0], in_=yt_h[c, 0:128])
        nc.sync.dma_start(out=ytt[1], in_=yt_h[c, 128:256])
        nc.sync.dma_start(out=ytt[2], in_=yt_h[c, 256:296])
        psY = ps.tile([P, FD], F32, tag="psY", name=f"psY{c}")
        psX = ps.tile([P, FD], F32, tag="psX", name=f"psX{c}")
        for i in range(NCH):
            osl = slice(i * WS, (i + 1) * WS)
            for kp in range(3):
                src = (i - 1 + kp) % NCH
                nc.tensor.matmul(
                    psY[:, osl], lhsT=ty[kp],
                    rhs=yn[c][:, src * WS:(src + 1) * WS],
                    start=(kp == 0), stop=(kp == 2))
        on_Y(psY)
        for i in range(NCH):
            osl = slice(i * WS, (i + 1) * WS)
            hsl = slice(i * P, (i + 1) * P)
            for kp in range(3):
                nc.tensor.matmul(
                    psX[:, osl], lhsT=ytt[kp][:, hsl], rhs=tx[kp],
                    start=(kp == 0), stop=(kp == 2))
        on_X(psX)

    def copy_to(dst):
        def f(psrc):
            nc.scalar.copy(dst, psrc)
        return f

    # ---------------- early DVE (dep: m planes) ----------------------------
    m01, m10, m00, m11 = yn[1], yn[2], yn[0], yn[3]
    S = longp("S")
    nc.gpsimd.tensor_add(S, m01, m10)    # Pool
    D = longp("D")
    vsub(D, m00, m11)
    T = longp("T")
    nc.gpsimd.tensor_add(T, m00, m11)    # Pool

    # ---------------- early ACT (squares feed the G2 critical path) --------
    m01sq = plane("m01sq")
    ACT(m01sq, m01, AF.Square)
    m10sq = plane("m10sq")
    ACT(m10sq, m10, AF.Square)
    hD2 = plane("hD2")
    ACT(hD2, D, AF.Square, scale=float(np.sqrt(0.5)))

    s_a = plane("s_a")
    vadd(s_a, m01sq, hD2)
    G2 = plane("G2")
    vadd(G2, s_a, m10sq)

    sc_t = tiny.tile([P, 2], F32, tag="sc", name="sc")
    nc.sync.dma_start(out=sc_t, in_=sc_h[:])
    sU_vec = sc_t[:, 0:1]
    sH_vec = sc_t[:, 1:2]

    g = plane("g", tag="g", bufs=1, dt=F32)
    ACT(g, G2, AF.Sqrt)
    rsG = plane("rsG", tag="rsG", bufs=1, dt=F32)
    nc.vector.reciprocal_approx_fast(out=rsG, in_=g)
    # ---------------- conv 5 (v0) ------------------------------------------
    E00 = longp("E00")
    Xv0 = plane("Xv0")
    conv_channel(5, copy_to(E00), copy_to(Xv0))

    # remaining input planes (issued on SP after conv5's yt loads)
    load(4, pln)    # c field
    load(7, pln)    # gamma (pre-scaled by cad2 on host)

    # ---------------- conv 4 (c field) early: feeds DVE during phase-5 wait
    sY4 = plane("sY4", tag="scpy", bufs=3)
    sX4 = plane("sX4", tag="scpy", bufs=3)
    conv_channel(4, copy_to(sY4), copy_to(sX4))
    t1_4 = plane("t1_4", tag="advt", bufs=3)
    vmul(t1_4, yn[5], sY4)
    t2_4 = plane("t2_4", tag="advt", bufs=3)
    pmul(t2_4, yn[6], sX4)                # Pool
    adv4 = plane("adv4", tag="t3", bufs=2)
    vadd(adv4, t1_4, t2_4)

    # ---------------- conv 6 (v1) ------------------------------------------
    Yv1 = plane("Yv1")
    E11 = longp("E11")
    conv_channel(6, copy_to(Yv1), copy_to(E11))

    u = plane("u", tag="u", bufs=1)
    ACT(u, g, AF.Copy, bias=1.0, scale=sU_vec)
    cfld = yn[4]
    cc = longp("cc")
    TS(cc, cfld, -myo2, myo1, OP.mult, OP.add)

    # ---------------- velocity combos --------------------------------------
    W2 = plane("W2")
    vsub(W2, Yv1, Xv0)
    A2p = longp("A2p")
    vadd(A2p, Xv0, Yv1)
    trE = longp("trE")
    vadd(trE, E00, E11)
    Bm = plane("Bm")
    vsub(Bm, E00, E11)
    hW2 = plane("hW2")
    ACT(hW2, W2, AF.Copy, scale=0.5)
    mt = longp("mt")
    ACT(mt, T, AF.Copy, scale=float(myo3))

    Q2 = longp("Q2")
    vmul(Q2, hW2, D)
    So = longp("So")
    pmul(So, hW2, S)                      # Pool
    p1 = plane("p1")
    vmul(p1, D, Bm)
    p2 = plane("p2")
    vmul(p2, A2p, S)
    devE2 = plane("devE2")
    vadd(devE2, p1, p2)
    habs = plane("habs")
    ACT(habs, devE2, AF.Abs, scale=sH_vec)
    h = plane("h")
    vmul(h, habs, rsG)
    uc = longp("uc")
    vmul(uc, u, cc)
    hc = longp("hc")
    vmul(hc, h, cc)
    trEb = plane("trEb")
    TS(trEb, trE, cad1, cad0, OP.mult, OP.subtract)

    # ---------------- cdot assembly ----------------------------------------
    w1c = plane("w1c")
    vmul(w1c, cfld, trEb)
    sc1 = plane("sc1")
    vsub(sc1, w1c, adv4)
    cdot = outp.tile([P, FD], BF16, tag="out", name="cdot", bufs=2)
    vadd(cdot, sc1, yn[7])

    # ---------------- phase-B scalars --------------------------------------
    u2 = longp("u2")
    TS(u2, uc, 2.0, None, OP.mult)
    huc = plane("huc", tag="huc", bufs=1)
    TS(huc, uc, 0.5, None, OP.mult)
    trEpc = plane("trEpc")
    pmul(trEpc, uc, trE)                  # Pool
    rq = longp("rq")
    hcD = plane("hcD")
    pmul(hcD, hc, D)                      # Pool
    q = longp("q")
    TS(q, T, myo4, myo0, OP.mult, OP.subtract)
    vadd(rq, trEpc, q)
    qp = plane("qp")
    vadd(qp, hcD, q)
    qm = plane("qm")
    vsub(qm, q, hcD)
    a00 = plane("a00")
    pmul(a00, u2, E00)                    # Pool
    r00 = longp("r00")
    vadd(r00, a00, qp)
    a11 = plane("a11")
    pmul(a11, u2, E11)                    # Pool
    r11 = longp("r11")
    vadd(r11, a11, qm)
    ucA = plane("ucA")
    vmul(ucA, huc, A2p)
    hm01 = plane("hm01")
    vmul(hm01, hc, m01)
    Epc01 = longp("Epc01")
    vadd(Epc01, ucA, hm01)
    hm10 = plane("hm10")
    vmul(hm10, hc, m10)
    Epc10 = longp("Epc10")
    vadd(Epc10, ucA, hm10)

    def adv_copy(c, t3, t2mul):
        sY = plane(f"sY{c}", tag="scpy", bufs=3)
        sX = plane(f"sX{c}", tag="scpy", bufs=3)
        conv_channel(c, copy_to(sY), copy_to(sX))
        t1 = plane(f"t1_{c}", tag="advt", bufs=3)
        vmul(t1, yn[5], sY)
        t2 = plane(f"t2_{c}", tag="advt", bufs=3)
        t2mul(t2, yn[6], sX)
        vadd(t3, t1, t2)

    # ---------------- m-channel 1 -> o01 -----------------------------------
    t3_1 = plane("t3_1", tag="t3", bufs=2)
    adv_copy(1, t3_1, pmul)
    w1 = plane("w1")
    vmul(w1, Epc01, T)
    mq01 = plane("mq01")
    pmul(mq01, m01, rq)                   # Pool
    preA1 = plane("preA1")
    vsub(preA1, Q2, t3_1)
    s01 = plane("s01")
    vadd(s01, w1, mq01)
    o01 = outp.tile([P, FD], BF16, tag="out", name="o01", bufs=2)
    vadd(o01, s01, preA1)

    # ---------------- m-channel 2 -> o10 -----------------------------------
    t3_2 = plane("t3_2", tag="t3", bufs=2)
    adv_copy(2, t3_2, pmul)
    w3 = plane("w3")
    pmul(w3, Epc10, T)                    # Pool
    mq10 = plane("mq10")
    pmul(mq10, m10, rq)                   # Pool
    preA2 = plane("preA2")
    vsub(preA2, Q2, t3_2)
    s10 = plane("s10")
    vadd(s10, w3, mq10)
    o10 = outp.tile([P, FD], BF16, tag="out", name="o10", bufs=2)
    vadd(o10, s10, preA2)

    # ---------------- m-channel 0 -> o00 -----------------------------------
    t3_0 = plane("t3_0", tag="t3", bufs=2)
    adv_copy(0, t3_0, vmul)
    x1 = plane("x1")
    vmul(x1, m01, Epc10)
    x2 = plane("x2")
    vmul(x2, m10, Epc01)
    X = plane("X", tag="X", bufs=1)
    vadd(X, x1, x2)
    mm00 = plane("mm00")
    vmul(mm00, m00, r00)

    # ---------------- m-channel 3 -> o11 -----------------------------------
    t3_3 = plane("t3_3", tag="t3", bufs=2)
    adv_copy(3, t3_3, vmul)
    mm11 = plane("mm11")
    vmul(mm11, m11, r11)

    # o00 = mm00 + X + (mt - t3_0 - So);  o11 = mm11 + X + (So - t3_3)
    e1 = plane("e1")
    vsub(e1, mt, t3_0)
    e2 = plane("e2")
    vsub(e2, e1, So)
    t00 = plane("t00")
    vadd(t00, mm00, X)
    o00 = outp.tile([P, FD], BF16, tag="out", name="o00", bufs=2)
    vadd(o00, t00, e2)
    f1 = plane("f1")
    vsub(f1, So, t3_3)
    t11 = plane("t11")
    vadd(t11, mm11, X)
    o11 = outp.tile([P, FD], BF16, tag="out", name="o11", bufs=2)
    vadd(o11, t11, f1)

    # output DMAs last on the SP queue: all input issues precede them
    nc.sync.dma_start(out=out_h[4], in_=cdot)
    nc.sync.dma_start(out=out_h[1], in_=o01)
    nc.sync.dma_start(out=out_h[2], in_=o10)
    nc.sync.dma_start(out=out_h[0], in_=o00)
    nc.sync.dma_start(out=out_h[3], in_=o11)


# ---------------------------------------------------------------------------
# host entry point
# ---------------------------------------------------------------------------

_CACHE = {}


def _get_nc(cad, myo):
    key = (tuple(np.asarray(cad, np.float64).tolist()),
           tuple(np.asarray(myo, np.float64).tolist()))
    if key not in _CACHE:
        nc = bacc.Bacc("TRN2", target_bir_lowering=False, debug=False,
                       num_devices=NCORES)
        _build(nc, *key)
        nc.compile()
        _CACHE[key] = nc
    return _CACHE[key]


def _make_in_maps(y, v, gamma_ds, cad):
    all7 = np.concatenate([y, v], axis=0).astype(np.float32)   # [7, H, W]
    ypad = np.pad(all7, ((0, 0), (0, 0), (R, R)), mode="reflect")
    ct = _build_consts()

    m4 = y[:4].astype(np.float32)
    m_norm = np.sqrt(np.sum(m4.astype(np.float64) ** 2, axis=0))
    m0 = float(np.mean(m_norm))
    sc = np.tile(np.array([[-0.5 / m0, 0.25 / m0]], np.float32), (P, 1))

    gam_s = (cad[2] * gamma_ds).astype(np.float32)
    all8 = np.concatenate([all7, gam_s[None]], axis=0)

    in_maps = []
    for core in range(NCORES):
        w0 = core * WS
        yn = _to_plane(all8[:, :, w0:w0 + WS]).astype(_BF)
        yt = np.ascontiguousarray(
            ypad[:, :, w0:w0 + WS + 2 * R].transpose(0, 2, 1)).astype(_BF)
        in_maps.append({"yn": yn, "yt": yt, "ct": ct, "sc": sc})
    return in_maps


def kernel(y, v, gamma_ds, cad_coefs, myo_coefs):
    y = np.asarray(y, np.float32)
    v = np.asarray(v, np.float32)
    gamma_ds = np.asarray(gamma_ds, np.float32)
    cad = np.maximum(np.asarray(cad_coefs, np.float32), 0)
    myo = np.maximum(np.asarray(myo_coefs, np.float32), 0)

    nc = _get_nc(cad, myo)
    in_maps = _make_in_maps(y, v, gamma_ds, cad)
    res = run_bass_kernel_spmd(nc, in_maps, core_ids=list(range(NCORES)))
    outs = [_from_plane(res.results[c]["out"].astype(np.float32))
            for c in range(NCORES)]
    return np.concatenate(outs, axis=-1)


# revision 20
# speedup vs baseline: 1.1129x; 1.1101x over previous
"""Trainium2 Bass kernel for nn_ClosedFlyLoop (8 NeuronCores, W-sharded).

Strategy (v4)
-------------
- W (AP) axis sharded across 8 cores; halos handled on the host during
  sharding, so all device work is local.
- m_0 (global mean of |m|) is computed on the HOST and shipped as two
  per-partition scalars (sU=-0.5/m_0, sH=0.25/m_0): removes the AllReduce
  (15+us fixed cost) and its serialization.
- All small constants (Toeplitz pieces + identity matrices) are packed into
  ONE DMA; input planes are loaded m/v-first so both the m-chain and the
  first convolutions start ~2us in.
- 14 Gaussian-derivative convolutions run on TensorE as banded-Toeplitz
  matmuls into full-plane [128,2048] PSUM tiles.
- The advection sign is folded into downstream subtracts / negated-identity
  matmuls, so v0/v1 are used unnegated (two fewer ops, one less plane).
- Elementwise 2x2 algebra is spread across engines by measured cost:
    * DVE: bf16 tensor_tensor at 2 elem/cycle (1127ns/plane-op).
    * Pool (gpsimd): ~11 independent tensor_tensor ops (4158ns each) chosen
      OFF the critical tail (side products + per-channel advection t2).
    * ACT: unary-affine ops + PSUM->SBUF copies of conv outputs.
    * PE: composes o00/o11 as sums of product planes via identity-matmul
      PSUM accumulation (214ns per [128,512] slice) after the convs release
      PSUM; ACT copies the sums back as bf16 for the output DMA.

Layout: each [1024, 256] per-core field is a "plane" [128, 2048] with
plane[p, a*256 + w] = field[a*128 + p, w]  (a = h-chunk index).
"""

import numpy as np
import ml_dtypes
from contextlib import ExitStack

import concourse.bass as bass
import concourse.bacc as bacc
import concourse.tile as tile
from concourse import mybir
from concourse.bass_utils import run_bass_kernel_spmd

F32 = mybir.dt.float32
BF16 = mybir.dt.bfloat16
AF = mybir.ActivationFunctionType
OP = mybir.AluOpType

H, W = 1024, 2048
NCORES = 8
WS = W // NCORES          # 256 columns per core
NCH = 8                   # h-chunks per core
P = 128
FD = NCH * WS             # 2048 plane free dim
R = 20                    # halo / pad radius
SIGMA = 5
D_AP = 2.27
D_DV = 2.27

_BF = ml_dtypes.bfloat16

# const pack layout (columns of the single [128, 1408] bf16 const tensor)
_C_TY = 0          # 3 x 128 cols  (tyA tyB tyC)
_C_TX = 384        # 2 x 256 cols  (txA txB)
_C_TXC = 896       # 256 cols      (txC padded to 128 partitions)
_C_ID = 1152       # 128 cols      identity
_C_NID = 1280      # 128 cols      negated identity
_C_COLS = 1408


def _gauss_d1_kernel():
    x = np.arange(-R, R + 1).astype(np.float64)
    phi = np.exp(-0.5 / SIGMA ** 2 * x ** 2)
    phi = phi / phi.sum()
    return ((-x / SIGMA ** 2) * phi).astype(np.float32)


def _build_consts():
    K = _gauss_d1_kernel().astype(np.float64)
    Ky = K / D_DV
    Kx = K / D_AP
    k = np.arange(128)[:, None]
    m = np.arange(128)[None, :]
    x = np.arange(WS)[None, :]

    def band(tmat, idx, taps):
        ok = (idx >= 0) & (idx <= 40)
        tmat[ok] = taps[idx[ok]]
        return tmat

    tyA = band(np.zeros((128, 128)), k - m - 108, Ky)
    tyB = band(np.zeros((128, 128)), k - m + 20, Ky)
    tyC = band(np.zeros((128, 128)), k - m + 148, Ky)
    txA = band(np.zeros((128, WS)), k - x, Kx)
    txB = band(np.zeros((128, WS)), 128 + k - x, Kx)
    txC = band(np.zeros((128, WS)), 256 + np.arange(128)[:, None] - x, Kx)
    ident = np.eye(128)
    pack = np.concatenate(
        [tyA, tyB, tyC, txA, txB, txC, ident, -ident], axis=1)
    assert pack.shape == (128, _C_COLS)
    return pack.astype(_BF)


def _to_plane(arr):
    """[..., H, WS] -> [..., 128, FD]"""
    sh = arr.shape[:-2]
    return (arr.reshape(*sh, NCH, P, WS).swapaxes(-3, -2)
            .reshape(*sh, P, FD))


def _from_plane(pl):
    sh = pl.shape[:-2]
    return (pl.reshape(*sh, P, NCH, WS).swapaxes(-3, -2)
            .reshape(*sh, H, WS))


# ---------------------------------------------------------------------------
# device kernel builder
# ---------------------------------------------------------------------------

def _build(nc, cad, myo):
    yn_h = nc.dram_tensor("yn", [8, P, FD], BF16, kind="ExternalInput")
    yt_h = nc.dram_tensor("yt", [7, WS + 2 * R, H], BF16, kind="ExternalInput")
    ct_h = nc.dram_tensor("ct", [P, _C_COLS], BF16, kind="ExternalInput")
    sc_h = nc.dram_tensor("sc", [P, 2], F32, kind="ExternalInput")
    out_h = nc.dram_tensor("out", [5, P, FD], BF16, kind="ExternalOutput")

    with tile.TileContext(nc, num_cores=NCORES) as tc:
        with ExitStack() as ctx:
            _body(ctx, tc, yn_h, yt_h, ct_h, sc_h, out_h, cad, myo)
    return nc


def _body(ctx, tc, yn_h, yt_h, ct_h, sc_h, out_h, cad, myo):
    nc = tc.nc
    cad0, cad1, cad2 = cad
    myo0, myo1, myo2, myo3, myo4 = myo

    pln = ctx.enter_context(tc.tile_pool(name="pln", bufs=1))
    lng = ctx.enter_context(tc.tile_pool(name="lng", bufs=1))
    ytp = ctx.enter_context(tc.tile_pool(name="ytp", bufs=1))
    tmpp = ctx.enter_context(tc.tile_pool(name="tmpp", bufs=5))
    outp = ctx.enter_context(tc.tile_pool(name="outp", bufs=1))
    ps = ctx.enter_context(tc.tile_pool(name="ps", bufs=1, space="PSUM"))
    tiny = ctx.enter_context(tc.tile_pool(name="tiny", bufs=1))

    def plane(name, pool=None, dt=BF16, tag=None, bufs=None):
        pool = pool if pool is not None else tmpp
        tag = tag if tag is not None else ("tmp" if pool is tmpp else name)
        return pool.tile([P, FD], dt, tag=tag, name=name, bufs=bufs)

    def longp(name, dt=BF16):
        return plane(name, lng, dt)

    TS = nc.vector.tensor_scalar
    ACT = nc.scalar.activation

    def vadd(out, a, b):
        nc.vector.tensor_add(out, a, b)

    def vsub(out, a, b):
        nc.vector.tensor_sub(out, a, b)

    def vmul(out, a, b):
        nc.vector.tensor_mul(out, a, b)

    def pmul(out, a, b):
        nc.gpsimd.tensor_mul(out, a, b)

    def psub(out, a, b):
        nc.gpsimd.tensor_sub(out, a, b)

    # ---------------- loads (order matters: SP queue is in-order) ----------
    ct = pln.tile([P, _C_COLS], BF16, tag="ct", name="ct")
    nc.sync.dma_start(out=ct, in_=ct_h[:])
    ty = [ct[:, _C_TY + 128 * j:_C_TY + 128 * (j + 1)] for j in range(3)]
    tx = [ct[:, _C_TX:_C_TX + WS], ct[:, _C_TX + WS:_C_TX + 2 * WS],
          ct[0:40, _C_TXC:_C_TXC + WS]]
    ident = ct[:, _C_ID:_C_ID + 128]
    nident = ct[:, _C_NID:_C_NID + 128]

    yn = [None] * 8

    def load(c, pool, tag=None):
        t = plane(f"yn{c}", pool, tag=tag, bufs=2 if tag else None)
        nc.sync.dma_start(out=t, in_=yn_h[c])
        yn[c] = t

    load(1, pln)            # m01 (squares first)
    load(5, tmpp, "vv")     # v0  (conv 5)
    load(0, pln)            # m00
    load(2, pln)            # m10
    load(6, tmpp, "vv")     # v1
    load(3, pln)            # m11

    # ---------------- conv machinery ----------------
    def conv_channel(c, on_Y, on_X):
        ytt = [ytp.tile([128, H], BF16, tag="ytk0", name=f"ytk0_{c}"),
               ytp.tile([128, H], BF16, tag="ytk1", name=f"ytk1_{c}"),
               ytp.tile([40, H], BF16, tag="ytk2", name=f"ytk2_{c}")]
        nc.sync.dma_start(out=ytt[0], in_=yt_h[c, 0:128])
        nc.sync.dma_start(out=ytt[1], in_=yt_h[c, 128:256])
        nc.sync.dma_start(out=ytt[2], in_=yt_h[c, 256:296])
        psY = ps.tile([P, FD], F32, tag="psY", name=f"psY{c}")
        psX = ps.tile([P, FD], F32, tag="psX", name=f"psX{c}")
        for i in range(NCH):
            osl = slice(i * WS, (i + 1) * WS)
            for kp in range(3):
                src = (i - 1 + kp) % NCH
                nc.tensor.matmul(
                    psY[:, osl], lhsT=ty[kp],
                    rhs=yn[c][:, src * WS:(src + 1) * WS],
                    start=(kp == 0), stop=(kp == 2))
        on_Y(psY)
        for i in range(NCH):
            osl = slice(i * WS, (i + 1) * WS)
            hsl = slice(i * P, (i + 1) * P)
            for kp in range(3):
                nc.tensor.matmul(
                    psX[:, osl], lhsT=ytt[kp][:, hsl], rhs=tx[kp],
                    start=(kp == 0), stop=(kp == 2))
        on_X(psX)

    def copy_to(dst):
        def f(psrc):
            nc.scalar.copy(dst, psrc)
        return f

    # PE output composition into a conv-PSUM slot (tag-shared; only used
    # after the convs release it), ACT-copied back as bf16 for the DMA.
    def compose(name, terms, tag):
        psO = ps.tile([P, FD], F32, tag=tag, name=f"pso_{name}")
        nterm = len(terms)
        for i in range(4):
            osl = slice(i * 512, (i + 1) * 512)
            for j, (pl, neg) in enumerate(terms):
                nc.tensor.matmul(
                    psO[:, osl], lhsT=(nident if neg else ident),
                    rhs=pl[:, osl], start=(j == 0), stop=(j == nterm - 1))
        o = outp.tile([P, FD], BF16, tag="out", name=name, bufs=2)
        nc.scalar.copy(o, psO)
        return o

    # ---------------- early DVE (dep: m planes) ----------------------------
    m01, m10, m00, m11 = yn[1], yn[2], yn[0], yn[3]
    S = longp("S")
    vadd(S, m01, m10)
    D = longp("D")
    vsub(D, m00, m11)
    T = longp("T")
    vadd(T, m00, m11)
    q = longp("q")
    TS(q, T, myo4, myo0, OP.mult, OP.subtract)

    # ---------------- early ACT (squares feed the G2 critical path) --------
    m01sq = plane("m01sq")
    ACT(m01sq, m01, AF.Square)
    m10sq = plane("m10sq")
    ACT(m10sq, m10, AF.Square)
    hD2 = plane("hD2")
    ACT(hD2, D, AF.Square, scale=float(np.sqrt(0.5)))

    s_a = plane("s_a")
    vadd(s_a, m01sq, hD2)
    G2 = plane("G2")
    vadd(G2, s_a, m10sq)

    # ---------------- conv 5 (v0) ------------------------------------------
    E00 = longp("E00")
    Xv0 = plane("Xv0")
    conv_channel(5, copy_to(E00), copy_to(Xv0))

    # remaining input planes (issued on SP after conv5's yt loads)
    load(4, pln)    # c field
    load(7, pln)    # gamma (pre-scaled by cad2 on host)
    sc_t = tiny.tile([P, 2], F32, tag="sc", name="sc")
    nc.sync.dma_start(out=sc_t, in_=sc_h[:])
    sU_vec = sc_t[:, 0:1]
    sH_vec = sc_t[:, 1:2]

    g = plane("g", tag="g", bufs=1, dt=F32)
    ACT(g, G2, AF.Sqrt)
    rsG = plane("rsG", tag="rsG", bufs=1, dt=F32)
    nc.vector.reciprocal_approx_fast(out=rsG, in_=g)
    u = plane("u", tag="u", bufs=1)
    ACT(u, g, AF.Copy, bias=1.0, scale=sU_vec)

    # ---------------- conv 6 (v1) ------------------------------------------
    Yv1 = plane("Yv1")
    E11 = longp("E11")
    conv_channel(6, copy_to(Yv1), copy_to(E11))

    cfld = yn[4]
    cc = longp("cc")
    TS(cc, cfld, -myo2, myo1, OP.mult, OP.add)

    # ---------------- velocity combos --------------------------------------
    W2 = plane("W2")
    vsub(W2, Yv1, Xv0)
    A2p = longp("A2p")
    vadd(A2p, Xv0, Yv1)
    trE = longp("trE")
    vadd(trE, E00, E11)
    Bm = plane("Bm")
    psub(Bm, E00, E11)                    # Pool
    hW2 = plane("hW2")
    ACT(hW2, W2, AF.Copy, scale=0.5)
    mt = longp("mt")
    ACT(mt, T, AF.Copy, scale=float(myo3))

    Q2 = longp("Q2")
    vmul(Q2, hW2, D)
    So = longp("So")
    pmul(So, hW2, S)                      # Pool
    p1 = plane("p1")
    vmul(p1, D, Bm)
    p2 = plane("p2")
    vmul(p2, A2p, S)
    devE2 = plane("devE2")
    vadd(devE2, p1, p2)
    habs = plane("habs")
    ACT(habs, devE2, AF.Abs, scale=sH_vec)
    h = plane("h")
    vmul(h, habs, rsG)
    uc = longp("uc")
    vmul(uc, u, cc)
    hc = longp("hc")
    vmul(hc, h, cc)
    trEb = plane("trEb")
    TS(trEb, trE, cad1, cad0, OP.mult, OP.subtract)

    # ---------------- c-channel conv + cdot --------------------------------
    sY4 = plane("sY4", tag="scpy", bufs=2)
    sX4 = plane("sX4", tag="scpy", bufs=2)
    conv_channel(4, copy_to(sY4), copy_to(sX4))
    t1_4 = plane("t1_4", tag="advt", bufs=2)
    vmul(t1_4, yn[5], sY4)
    t2_4 = plane("t2_4", tag="advt", bufs=2)
    pmul(t2_4, yn[6], sX4)                # Pool
    adv4 = plane("adv4", tag="t3", bufs=2)
    vadd(adv4, t1_4, t2_4)
    w1c = plane("w1c")
    vmul(w1c, cfld, trEb)
    sc1 = plane("sc1")
    vsub(sc1, w1c, adv4)
    cdot = outp.tile([P, FD], BF16, tag="out", name="cdot", bufs=2)
    vadd(cdot, sc1, yn[7])

    # ---------------- phase-B scalars --------------------------------------
    u2 = longp("u2")
    TS(u2, uc, 2.0, None, OP.mult)
    huc = plane("huc", tag="huc", bufs=1)
    TS(huc, uc, 0.5, None, OP.mult)
    trEpc = plane("trEpc")
    vmul(trEpc, uc, trE)
    rq = longp("rq")
    hcD = plane("hcD")
    vmul(hcD, hc, D)
    q = longp("q")
    TS(q, T, myo4, myo0, OP.mult, OP.subtract)
    vadd(rq, trEpc, q)
    qp = plane("qp")
    vadd(qp, hcD, q)
    qm = plane("qm")
    vsub(qm, q, hcD)
    a00 = plane("a00")
    vmul(a00, u2, E00)
    r00 = longp("r00")
    vadd(r00, a00, qp)
    a11 = plane("a11")
    vmul(a11, u2, E11)
    r11 = longp("r11")
    vadd(r11, a11, qm)
    ucA = plane("ucA")
    vmul(ucA, huc, A2p)
    hm01 = plane("hm01")
    pmul(hm01, hc, m01)                   # Pool
    Epc01 = longp("Epc01")
    vadd(Epc01, ucA, hm01)
    hm10 = plane("hm10")
    vmul(hm10, hc, m10)
    Epc10 = longp("Epc10")
    vadd(Epc10, ucA, hm10)

    def adv_copy(c, t3, t2mul):
        sY = plane(f"sY{c}", tag="scpy", bufs=2)
        sX = plane(f"sX{c}", tag="scpy", bufs=2)
        conv_channel(c, copy_to(sY), copy_to(sX))
        t1 = plane(f"t1_{c}", tag="advt", bufs=2)
        vmul(t1, yn[5], sY)
        t2 = plane(f"t2_{c}", tag="advt", bufs=2)
        t2mul(t2, yn[6], sX)
        vadd(t3, t1, t2)

    # ---------------- m-channel 1 -> o01 -----------------------------------
    t3_1 = plane("t3_1", tag="t3", bufs=2)
    adv_copy(1, t3_1, pmul)
    w1 = plane("w1")
    vmul(w1, Epc01, T)
    mq01 = plane("mq01")
    pmul(mq01, m01, rq)                   # Pool
    preA1 = plane("preA1")
    vsub(preA1, Q2, t3_1)
    s01 = plane("s01")
    vadd(s01, w1, mq01)
    o01 = outp.tile([P, FD], BF16, tag="out", name="o01", bufs=2)
    vadd(o01, s01, preA1)

    # ---------------- m-channel 2 -> o10 -----------------------------------
    t3_2 = plane("t3_2", tag="t3", bufs=2)
    adv_copy(2, t3_2, pmul)
    w3 = plane("w3")
    vmul(w3, Epc10, T)
    mq10 = plane("mq10")
    pmul(mq10, m10, rq)                   # Pool
    preA2 = plane("preA2")
    vsub(preA2, Q2, t3_2)
    s10 = plane("s10")
    vadd(s10, w3, mq10)
    o10 = outp.tile([P, FD], BF16, tag="out", name="o10", bufs=2)
    vadd(o10, s10, preA2)

    # ---------------- m-channel 0 -> o00 -----------------------------------
    t3_0 = plane("t3_0", tag="t3", bufs=2)
    adv_copy(0, t3_0, pmul)
    x1 = plane("x1")
    vmul(x1, m01, Epc10)
    x2 = plane("x2")
    pmul(x2, m10, Epc01)                  # Pool
    X = plane("X", tag="X", bufs=1)
    vadd(X, x1, x2)
    mm00 = plane("mm00")
    vmul(mm00, m00, r00)

    # ---------------- m-channel 3 -> o11 -----------------------------------
    t3_3 = plane("t3_3", tag="t3", bufs=2)
    adv_copy(3, t3_3, pmul)
    mm11 = plane("mm11")
    vmul(mm11, m11, r11)

    # PE composes the last two outputs once the convs release PSUM
    o00 = compose("o00", [(mm00, False), (X, False), (t3_0, True),
                          (mt, False), (So, True)], tag="psY")
    o11 = compose("o11", [(mm11, False), (X, False), (t3_3, True),
                          (So, False)], tag="psX")

    # output DMAs last on the SP queue: all input issues precede them
    nc.sync.dma_start(out=out_h[4], in_=cdot)
    nc.sync.dma_start(out=out_h[1], in_=o01)
    nc.sync.dma_start(out=out_h[2], in_=o10)
    nc.sync.dma_start(out=out_h[0], in_=o00)
    nc.sync.dma_start(out=out_h[3], in_=o11)


# ---------------------------------------------------------------------------
# host entry point
# ---------------------------------------------------------------------------

_CACHE = {}


def _get_nc(cad, myo):
    key = (tuple(np.asarray(cad, np.float64).tolist()),
           tuple(np.asarray(myo, np.float64).tolist()))
    if key not in _CACHE:
        nc = bacc.Bacc("TRN2", target_bir_lowering=False, debug=False,
                       num_devices=NCORES)
        _build(nc, *key)
        nc.compile()
        _CACHE[key] = nc
    return _CACHE[key]


def _make_in_maps(y, v, gamma_ds, cad):
    all7 = np.concatenate([y, v], axis=0).astype(np.float32)   # [7, H, W]
    ypad = np.pad(all7, ((0, 0), (0, 0), (R, R)), mode="reflect")
    ct = _build_consts()

    m4 = y[:4].astype(np.float32)
    m_norm = np.sqrt(np.sum(m4.astype(np.float64) ** 2, axis=0))
    m0 = float(np.mean(m_norm))
    sc = np.tile(np.array([[-0.5 / m0, 0.25 / m0]], np.float32), (P, 1))

    gam_s = (cad[2] * gamma_ds).astype(np.float32)
    all8 = np.concatenate([all7, gam_s[None]], axis=0)

    in_maps = []
    for core in range(NCORES):
        w0 = core * WS
        yn = _to_plane(all8[:, :, w0:w0 + WS]).astype(_BF)
        yt = np.ascontiguousarray(
            ypad[:, :, w0:w0 + WS + 2 * R].transpose(0, 2, 1)).astype(_BF)
        in_maps.append({"yn": yn, "yt": yt, "ct": ct, "sc": sc})
    return in_maps


def kernel(y, v, gamma_ds, cad_coefs, myo_coefs):
    y = np.asarray(y, np.float32)
    v = np.asarray(v, np.float32)
    gamma_ds = np.asarray(gamma_ds, np.float32)
    cad = np.maximum(np.asarray(cad_coefs, np.float32), 0)
    myo = np.maximum(np.asarray(myo_coefs, np.float32), 0)

    nc = _get_nc(cad, myo)
    in_maps = _make_in_maps(y, v, gamma_ds, cad)
    res = run_bass_kernel_spmd(nc, in_maps, core_ids=list(range(NCORES)))
    outs = [_from_plane(res.results[c]["out"].astype(np.float32))
            for c in range(NCORES)]
    return np.concatenate(outs, axis=-1)
